# revision 5
# baseline (speedup 1.0000x reference)
"""Conv2d(128->256, 3x3, pad=1) + sync-BatchNorm(train) + ReLU on 8 TRN2 cores.

Strategy (data-parallel, hardcoded for x:[32,128,56,56] w:[256,128,3,3]):
  - Shard batch 32 -> 4 images/core across 8 cores.
  - Host pre-pads x to 58x58, casts x/w to bf16, pre-transposes weights to
    [Cin, o_tile, o, tap] so every device DMA is contiguous.
  - Conv = implicit GEMM: Cin=128 is the partition/contraction dim; each 3x3 tap
    is one bf16 matmul ([128,128] weights x [128,448] shifted-image view)
    accumulated in fp32 PSUM. Output rows in 7 groups of 8 rows (8*56=448
    <= 512 fp32 PSUM bank); chunks of 4+3 groups, tap-major inside a chunk so
    one LDWEIGHTS covers 3-4 matmuls.
  - BN train-mode: conv bias cancels exactly ((y+b) - mean(y+b) == y - mean(y)).
    Per-channel partials: sum(y) folds into the PSUM-evacuating ACT Copy
    (accum_out); sum(y^2) is a DVE scalar_tensor_tensor over the SBUF copy so
    PSUM is released by the Copy alone.
  - Sync-BN: stats all-reduced per 128-channel otile. AR(o=0) triggers after
    o=0 convs and completes under o=1 convs; AR(o=1) triggers after the last
    evac and hides under o=0's normalize+store. A dummy warmup AR at kernel
    start syncs cores / warms the collective path under the input DMA.
  - Final: out = Relu(y*scale + shift), one fused ACT per (otile, image, chunk),
    DMA'd straight to DRAM right behind each ACT.
"""

import os

import numpy as np
import ml_dtypes

import concourse.bass as bass
import concourse.mybir as mybir
import concourse.tile as tile
from concourse import bacc

F32 = mybir.dt.float32
BF16 = mybir.dt.bfloat16

N_CORES = 8
IMGS = 4            # images per core
CIN = 128
COUT = 256
H = W = 56
HP = WP = 58        # padded
NG = 7              # row-groups per image (8 rows each)
RG = 8              # rows per group
GROUP = RG * W      # 448
BANK = 512          # fp32 elems per PSUM bank
EPS = 1e-5
COUNT = float(32 * H * W)   # global BN element count per channel

AF = mybir.ActivationFunctionType
ALU = mybir.AluOpType

CHUNKS = [(0, 4), (4, 3)]   # (first group, n groups) -> 4+3 PSUM banks


def build_nc() -> bass.Bass:
    no_ar = bool(os.environ.get("CONVACT_NO_AR"))
    # Bacc (not raw Bass): its compile pipeline legalizes semaphore waits
    # (TRN2 allows at most one wait per instruction; matmul waits move to
    # ldweights / event-semaphore instructions).
    nc = bacc.Bacc()
    xp_d = nc.declare_dram_parameter("xp", [IMGS, CIN, HP, WP], BF16, isOutput=False)
    wt_d = nc.declare_dram_parameter("wt", [CIN, 2, 128, 9], BF16, isOutput=False)
    gb_d = nc.declare_dram_parameter("gb", [128, 4], F32, isOutput=False)
    out_d = nc.declare_dram_parameter("out", [IMGS, COUT, H, W], F32, isOutput=True)

    with tile.TileContext(nc) as tc:
        with (
            tc.tile_pool(name="const", bufs=1) as cpool,
            tc.tile_pool(name="psum", bufs=2, space="PSUM") as ppool,
            tc.tile_pool(name="scrp", bufs=2) as spool,
            tc.tile_pool(name="dram", bufs=1, space="DRAM") as dpool,
        ):
            Wt = cpool.tile([128, 2, 128, 9], BF16)
            GB = cpool.tile([128, 4], F32)
            X = cpool.tile([128, IMGS, HP, WP], BF16)
            Y = cpool.tile([128, 2, IMGS, NG, GROUP], F32)
            Ssum = cpool.tile([128, 2, IMGS * 2], F32)
            Ssq = cpool.tile([128, 2, IMGS * 2], F32)
            ST = cpool.tile([128, 2, 2], F32)    # packed (sum, sumsq) per otile
            G = cpool.tile([128, 2, 2], F32)     # post-AR global (sum, sumsq)
            mean = cpool.tile([128, 2], F32)
            e2 = cpool.tile([128, 2], F32)
            msq = cpool.tile([128, 2], F32)
            var = cpool.tile([128, 2], F32)
            std = cpool.tile([128, 2], F32)
            inv = cpool.tile([128, 2], F32)
            sc = cpool.tile([128, 2], F32)
            sh = cpool.tile([128, 2], F32)
            epsT = cpool.tile([128, 1], F32)
            warm = cpool.tile([128, 1], F32)
            bnc_in = [
                dpool.tile([128, 2], F32, name=f"bnc_in{i}") for i in range(2)
            ]
            bnc_out = [
                dpool.tile([128, 2], F32, name=f"bnc_out{i}") for i in range(2)
            ]
            warm_in = dpool.tile([128, 1], F32)
            warm_out = dpool.tile([128, 1], F32)

            # ---- warmup collective: syncs cores + warms CC path, hidden
            # under the input DMAs / first convs ----
            nc.vector.memset(epsT[:, :], EPS)
            nc.vector.memset(warm[:, :], 0.0)
            nc.gpsimd.dma_start(warm_in[:, :], warm[:, :])
            if not no_ar:
                nc.gpsimd.collective_compute(
                    "AllReduce",
                    ALU.add,
                    replica_groups=[list(range(N_CORES))],
                    ins=[warm_in.opt()],
                    outs=[warm_out.opt()],
                )

            # ---- loads: critical-path first (o=0 weights, image 0) ----
            nc.scalar.dma_start(Wt[:, 0], wt_d[:, 0])
            nc.sync.dma_start(X[:, 0], xp_d[0])
            nc.scalar.dma_start(GB[:, :], gb_d[:, :])
            nc.scalar.dma_start(Wt[:, 1], wt_d[:, 1])
            nc.sync.dma_start(X[:, 1], xp_d[1])
            nc.gpsimd.dma_start(X[:, 2], xp_d[2])
            nc.gpsimd.dma_start(X[:, 3], xp_d[3])

            def conv_chunk(o, n, ci):
                g0, ngr = CHUNKS[ci]
                ps = ppool.tile([128, 4, BANK], F32, tag="ps")
                # tap-major: one ldweights per tap serves ngr matmuls
                for t in range(9):
                    kh, kw = divmod(t, 3)
                    for gg in range(ngr):
                        g = g0 + gg
                        rhs = X[:, n, g * RG + kh : g * RG + kh + RG, kw : kw + W]
                        nc.tensor.matmul(
                            ps[:, gg, 0:GROUP],
                            Wt[:, o, :, t],
                            rhs,
                            start=(t == 0),
                            stop=(t == 8),
                        )
                col = n * 2 + ci
                ysl = Y[:, o, n, g0 : g0 + ngr, :]
                # evacuate PSUM -> Y and fold sum(y) into the same ACT op;
                # PSUM is released by this Copy alone.
                nc.scalar.activation(
                    ysl,
                    ps[:, 0:ngr, 0:GROUP],
                    AF.Copy,
                    accum_out=Ssum[:, o, col : col + 1],
                )
                # sum(y^2) on DVE from the SBUF copy
                scr = spool.tile([128, 4, GROUP], F32, tag="scr")
                nc.vector.scalar_tensor_tensor(
                    scr[:, 0:ngr, :],
                    ysl,
                    1.0,
                    ysl,
                    ALU.mult,
                    ALU.mult,
                    accum_out=Ssq[:, o, col : col + 1],
                )

            def stats_trigger(o):
                # pack local (sum, sumsq) and kick off the per-otile AllReduce
                nc.vector.reduce_sum(
                    ST[:, o, 0:1], Ssum[:, o : o + 1, :], axis=mybir.AxisListType.X
                )
                nc.vector.reduce_sum(
                    ST[:, o, 1:2], Ssq[:, o : o + 1, :], axis=mybir.AxisListType.X
                )
                nc.gpsimd.dma_start(bnc_in[o][:, :], ST[:, o, :])
                if no_ar:
                    nc.gpsimd.dma_start(bnc_out[o][:, :], bnc_in[o][:, :])
                else:
                    nc.gpsimd.collective_compute(
                        "AllReduce",
                        ALU.add,
                        replica_groups=[list(range(N_CORES))],
                        ins=[bnc_in[o].opt()],
                        outs=[bnc_out[o].opt()],
                    )

            def stats_finalize(o):
                # global stats -> per-channel scale/shift for this otile
                nc.gpsimd.dma_start(G[:, o, :], bnc_out[o][:, :])
                inv_cnt = (N_CORES if no_ar else 1.0) / COUNT
                osl = slice(o, o + 1)
                nc.vector.tensor_scalar_mul(mean[:, osl], G[:, o, 0:1], inv_cnt)
                nc.vector.tensor_scalar_mul(e2[:, osl], G[:, o, 1:2], inv_cnt)
                nc.vector.tensor_mul(msq[:, osl], mean[:, osl], mean[:, osl])
                nc.vector.tensor_sub(var[:, osl], e2[:, osl], msq[:, osl])
                nc.scalar.activation(std[:, osl], var[:, osl], AF.Sqrt, bias=epsT[:, 0:1])
                nc.vector.reciprocal(inv[:, osl], std[:, osl])
                nc.vector.tensor_mul(sc[:, osl], GB[:, o : o + 1], inv[:, osl])
                nc.vector.tensor_mul(msq[:, osl], mean[:, osl], sc[:, osl])
                nc.vector.tensor_sub(sh[:, osl], GB[:, 2 + o : 3 + o], msq[:, osl])

            def norm_store(o, n, ci, engine):
                g0, ngr = CHUNKS[ci]
                ysl = Y[:, o, n, g0 : g0 + ngr, :]
                nc.scalar.activation(
                    ysl,
                    ysl,
                    AF.Relu,
                    bias=sh[:, o : o + 1],
                    scale=sc[:, o : o + 1],
                )
                engine.dma_start(
                    out_d[
                        n, o * 128 : (o + 1) * 128, g0 * RG : (g0 + ngr) * RG, :
                    ].rearrange("p h w -> p (h w)"),
                    ysl.rearrange("p a b -> p (a b)"),
                )

            # ---- o=0 convs, then AR(0) trigger ----
            for n in range(IMGS):
                conv_chunk(0, n, 0)
                conv_chunk(0, n, 1)
            stats_trigger(0)

            # ---- o=1 convs (AR(0) completes underneath), then AR(1) ----
            for n in range(IMGS):
                conv_chunk(1, n, 0)
                conv_chunk(1, n, 1)
            stats_trigger(1)

            # ---- o=0 normalize+store: hides AR(1) latency ----
            stats_finalize(0)
            for n in range(IMGS):
                norm_store(0, n, 0, nc.sync)
                norm_store(0, n, 1, nc.sync)

            # ---- o=1 normalize+store ----
            stats_finalize(1)
            for n in range(IMGS):
                norm_store(1, n, 0, nc.gpsimd)
                norm_store(1, n, 1, nc.gpsimd)
    return nc


_CACHE: dict = {}


def _get_nc() -> bass.Bass:
    if "nc" not in _CACHE:
        nc = build_nc()
        # Bacc.finalize runs the compile pipeline (wait legalization, register
        # allocation, nop fusion) - required before handing BIR to walrus.
        nc.finalize()
        _CACHE["nc"] = nc
    return _CACHE["nc"]


def _prep_inputs(x, weight, gamma, beta):
    x = np.ascontiguousarray(np.asarray(x, dtype=np.float32))
    w = np.asarray(weight, dtype=np.float32)
    gamma = np.asarray(gamma, dtype=np.float32)
    beta = np.asarray(beta, dtype=np.float32)

    B = x.shape[0]
    per = B // N_CORES
    xp = np.zeros((B, CIN, HP, WP), ml_dtypes.bfloat16)
    xp[:, :, 1 : 1 + H, 1 : 1 + W] = x.astype(ml_dtypes.bfloat16)
    wt = np.ascontiguousarray(
        w.transpose(1, 0, 2, 3).reshape(CIN, 2, 128, 9).astype(ml_dtypes.bfloat16)
    )
    gb = np.ascontiguousarray(
        np.stack([gamma[:128], gamma[128:], beta[:128], beta[128:]], axis=1)
    )
    return [
        {"xp": xp[c * per : (c + 1) * per], "wt": wt, "gb": gb}
        for c in range(N_CORES)
    ]


def run(x, weight, bias=None, gamma=None, beta=None, trace=False, **kw):
    """Full-input entry; returns (out, BassKernelResults)."""
    from concourse.bass_utils import run_bass_kernel_spmd

    in_maps = _prep_inputs(x, weight, gamma, beta)
    res = run_bass_kernel_spmd(
        _get_nc(), in_maps, list(range(N_CORES)), trace=trace, **kw
    )
    out = np.concatenate([res.results[c]["out"] for c in range(N_CORES)], axis=0)
    return out, res


def kernel(x, weight, bias=None, gamma=None, beta=None):
    out, _ = run(x, weight, bias=bias, gamma=gamma, beta=beta, trace=False)
    return out
